# revision 19
# baseline (speedup 1.0000x reference)
"""DeltaRule (diagonal-state linear attention) Bass kernel for 8 TRN2 cores.

Problem: nn_DeltaRule_20194936225992
  B=4, S=2048, H_DIM=1024, N_HEADS=16, HEAD_DIM=64.
  q/k/v/b projections, phi = elu+1, per-(b,h,d) scalar linear recurrence
      s_t = (1 - b_t*pk_t^2) * s_{t-1} + b_t*v_t*pk_t ;  y_t = s_t * pq_t
  out = y @ Wo.T + bo

Sharding: core = (batch b, head-group hg) with hg covering 8 heads.
Each core computes its partial O-projection (contraction over its 512
lanes); host sums the two head-group partials per batch, transposes
[o,t] -> [t,o] and adds bo.

Design notes (fp16 everywhere):
  - All matmul operands and elementwise intermediates are float16: PE runs
    fp16 at the same 1 cycle/row as bf16, DVE gets its 2x packed mode, and
    fp16's 10 mantissa bits keep end-to-end rel-err ~1.4e-3 (vs 1.1e-2 bf16).
  - phi(u) = elu(u)+1 = min(exp(u),1) + relu(u).  u = x@W is bounded (~3.7)
    so exp(u) cannot overflow fp16.  Two ACT ops (Relu, Exp, both reading
    PSUM directly with the bias folded in) + one DVE scalar_tensor_tensor.
  - v-bias folded into the ACT PSUM->SBUF copy (no ones-row matmul).
  - Gate math w=pk*b, g=pk*w, a=1-g, c=v*w, y=s*pq on DVE in fp16
    (tensor_tensor 2x mode / tensor_scalar 4x mode); scan in fp32 state.
  - O-projection PSUM->SBUF copies alternate ACT/DVE; O-proj of chunk c is
    emitted inside chunk c+1 so the PE never waits on the y's it just made.
  - Chunk 0 is phase-split (all-k, all-v+scan, all-q) with the DMA stream
    ordered to match consumption, because the first ~15us are HBM-paced.
  - The sigmoid gate b is computed on the host (0.4% of total FLOPs) and
    DMA'd pre-broadcast per lane.
"""

import os
import sys

for _p in ("/opt/trn_rl_repo", os.path.expanduser("~/.axon_site/_ro/trn_rl_repo")):
    if os.path.isdir(_p) and _p not in sys.path:
        sys.path.insert(0, _p)

import numpy as np  # noqa: E402

import concourse.bass as bass  # noqa: E402
import concourse.tile as tile  # noqa: E402
from concourse import bacc, mybir  # noqa: E402
from concourse.bass import ts  # noqa: E402
from concourse.bass_utils import run_bass_kernel_spmd  # noqa: E402

# problem constants (hardcoded per task rules)
B, S, H_DIM, N_HEADS, HEAD_DIM = 4, 2048, 1024, 16, 64
P = 128
NCORES = 8
HG = 2                      # head groups
J = 512                     # lanes per core  (8 heads * 64)
JT = J // P                 # 4 j-tiles
DT = H_DIM // P             # 8 contraction tiles
HPC = N_HEADS // HG         # 8 heads per core
TC = 512
NCH = S // TC

F32 = mybir.dt.float32
F16 = mybir.dt.float16
AF = mybir.ActivationFunctionType
M = mybir.AluOpType

# engine for the O-projection PSUM->SBUF copies: "mix" (alternate ACT/DVE)
# or "act" (all ACT).  (GpSimd/Pool cannot access PSUM on TRN2.)
O_COPY_ENG = os.environ.get("DELTA_OCOPY", "mix")


def build_nc():
    nc = bacc.Bacc(trn_type="TRN2", target_bir_lowering=False, debug=False)

    # per-core inputs; x tensors host-packed as [p, chunk, dt, t_in_chunk]
    xq = nc.dram_tensor("xq", [P, NCH, DT, TC], F16, kind="ExternalInput").ap()
    xk = nc.dram_tensor("xk", [P, NCH, DT, TC], F16, kind="ExternalInput").ap()
    xv = nc.dram_tensor("xv", [P, NCH, DT, TC], F16, kind="ExternalInput").ap()
    bbb = nc.dram_tensor("bbb", [P, NCH, JT, TC], F16, kind="ExternalInput").ap()
    wq = nc.dram_tensor("wq", [H_DIM, J], F16, kind="ExternalInput").ap()
    wk = nc.dram_tensor("wk", [H_DIM, J], F16, kind="ExternalInput").ap()
    wv = nc.dram_tensor("wv", [H_DIM, J], F16, kind="ExternalInput").ap()
    wo = nc.dram_tensor("wo", [J, H_DIM], F16, kind="ExternalInput").ap()
    bq = nc.dram_tensor("bq", [P, JT], F32, kind="ExternalInput").ap()
    bk = nc.dram_tensor("bk", [P, JT], F32, kind="ExternalInput").ap()
    bv = nc.dram_tensor("bv", [P, JT], F32, kind="ExternalInput").ap()
    out = nc.dram_tensor("out", [H_DIM, S], F16, kind="ExternalOutput").ap()
    out_r = out.rearrange("(dt p) t -> p dt t", p=P)

    from contextlib import ExitStack

    with tile.TileContext(nc) as tcx, ExitStack() as ctx:
        wpool = ctx.enter_context(tcx.tile_pool(name="weights", bufs=1))
        xpool = ctx.enter_context(tcx.tile_pool(name="xin", bufs=2))
        ipool = ctx.enter_context(tcx.tile_pool(name="inter", bufs=2))
        spool = ctx.enter_context(tcx.tile_pool(name="scan", bufs=2))
        opool = ctx.enter_context(tcx.tile_pool(name="osb", bufs=2))
        pproj = ctx.enter_context(tcx.tile_pool(name="pproj", bufs=5, space="PSUM"))
        po = ctx.enter_context(tcx.tile_pool(name="po", bufs=3, space="PSUM"))

        # --- persistent weights / constants ---
        wq_sb = wpool.tile([P, DT, J], F16, tag="wq")
        wk_sb = wpool.tile([P, DT, J], F16, tag="wk")
        wv_sb = wpool.tile([P, DT, J], F16, tag="wv")
        wo_sb = wpool.tile([P, JT, H_DIM], F16, tag="wo")
        bq_sb = wpool.tile([P, JT], F32, tag="bq")
        bk_sb = wpool.tile([P, JT], F32, tag="bk")
        bv_sb = wpool.tile([P, JT], F32, tag="bv")

        # k weights first (halved so the first matmuls start early)
        wk_r = wk.rearrange("(dt p) j -> p dt j", p=P)
        nc.sync.dma_start(out=wk_sb[:, 0:4, :], in_=wk_r[:, 0:4, :])
        nc.sync.dma_start(out=wk_sb[:, 4:8, :], in_=wk_r[:, 4:8, :])

        s_prev = [None] * JT   # last-chunk scan state tile per lane-tile
        y_prev = None          # previous chunk's y tiles (deferred O-proj)

        def emit_o_proj(c, ys, split_dma=False):
            osb = opool.tile([P, DT, TC], F16, tag="osb")
            for ot in range(DT):
                pso = po.tile([P, TC], F32, tag="po")
                for lt in range(JT):
                    nc.tensor.matmul(
                        out=pso[:], lhsT=wo_sb[:, lt, ts(ot, P)], rhs=ys[lt][:],
                        start=(lt == 0), stop=(lt == JT - 1),
                    )
                # alternate copies across ACT and DVE so neither serializes
                if ot % 2 == 0 and O_COPY_ENG != "act":
                    nc.vector.tensor_scalar(
                        out=osb[:, ot, :], in0=pso[:],
                        scalar1=0.0, scalar2=None, op0=M.add,
                    )
                else:
                    nc.scalar.copy(out=osb[:, ot, :], in_=pso[:])
                if split_dma:
                    nc.sync.dma_start(out=out_r[:, ot, ts(c, TC)],
                                      in_=osb[:, ot, :])
            if not split_dma:
                nc.sync.dma_start(out=out_r[:, :, ts(c, TC)], in_=osb[:])

        def emit_k(c, lt, xk_c):
            jsl = ts(lt, P)
            psk = pproj.tile([P, TC], F32, tag="proj")
            for d in range(DT):
                nc.tensor.matmul(
                    out=psk[:], lhsT=wk_sb[:, d, jsl], rhs=xk_c[:, d, :],
                    start=(d == 0), stop=(d == DT - 1),
                )
            rk = ipool.tile([P, TC], F16, tag="relu")
            nc.scalar.activation(out=rk[:], in_=psk[:], func=AF.Relu,
                                 bias=bk_sb[:, lt:lt + 1])
            ek = ipool.tile([P, TC], F16, tag="ex")
            nc.scalar.activation(out=ek[:], in_=psk[:], func=AF.Exp,
                                 bias=bk_sb[:, lt:lt + 1])
            pk = ipool.tile([P, TC], F16, tag=f"pk{lt}")
            nc.vector.scalar_tensor_tensor(
                out=pk[:], in0=ek[:], scalar=1.0, in1=rk[:],
                op0=M.min, op1=M.add,
            )
            return pk

        def emit_v_scan(c, lt, xv_c, bb_c, pk):
            jsl = ts(lt, P)
            psv = pproj.tile([P, TC], F32, tag="proj")
            for d in range(DT):
                nc.tensor.matmul(
                    out=psv[:], lhsT=wv_sb[:, d, jsl], rhs=xv_c[:, d, :],
                    start=(d == 0), stop=(d == DT - 1),
                )
            vsb = ipool.tile([P, TC], F16, tag="vsb")
            nc.scalar.activation(out=vsb[:], in_=psv[:], func=AF.Identity,
                                 bias=bv_sb[:, lt:lt + 1])
            w = ipool.tile([P, TC], F16, tag="w")
            nc.vector.tensor_tensor(out=w[:], in0=pk[:], in1=bb_c[:, lt, :], op=M.mult)
            g = ipool.tile([P, TC], F16, tag="g")
            nc.vector.tensor_tensor(out=g[:], in0=pk[:], in1=w[:], op=M.mult)
            a = ipool.tile([P, TC], F16, tag="a")
            nc.vector.tensor_scalar(out=a[:], in0=g[:], scalar1=-1.0,
                                    scalar2=1.0, op0=M.mult, op1=M.add)
            cc = ipool.tile([P, TC], F16, tag="cc")
            nc.vector.tensor_tensor(out=cc[:], in0=vsb[:], in1=w[:], op=M.mult)
            s_new = spool.tile([P, TC], F16, tag=f"s{lt}")
            init = 0.0 if c == 0 else s_prev[lt][:, TC - 1:TC]
            nc.vector.tensor_tensor_scan(
                out=s_new[:], data0=a[:], data1=cc[:], initial=init,
                op0=M.mult, op1=M.add,
            )
            s_prev[lt] = s_new
            return s_new

        def emit_q_y(c, lt, xq_c, s_new):
            jsl = ts(lt, P)
            psq = pproj.tile([P, TC], F32, tag="proj")
            for d in range(DT):
                nc.tensor.matmul(
                    out=psq[:], lhsT=wq_sb[:, d, jsl], rhs=xq_c[:, d, :],
                    start=(d == 0), stop=(d == DT - 1),
                )
            rq = ipool.tile([P, TC], F16, tag="relu")
            nc.scalar.activation(out=rq[:], in_=psq[:], func=AF.Relu,
                                 bias=bq_sb[:, lt:lt + 1])
            eq = ipool.tile([P, TC], F16, tag="ex")
            nc.scalar.activation(out=eq[:], in_=psq[:], func=AF.Exp,
                                 bias=bq_sb[:, lt:lt + 1])
            pq = ipool.tile([P, TC], F16, tag="pq")
            nc.vector.scalar_tensor_tensor(
                out=pq[:], in0=eq[:], scalar=1.0, in1=rq[:],
                op0=M.min, op1=M.add,
            )
            y = spool.tile([P, TC], F16, tag=f"y{lt}")
            nc.vector.tensor_tensor(out=y[:], in0=s_new[:], in1=pq[:], op=M.mult)
            return y

        for c in range(NCH):
            xk_c = xpool.tile([P, DT, TC], F16, tag="xk")
            xv_c = xpool.tile([P, DT, TC], F16, tag="xv")
            bb_c = xpool.tile([P, JT, TC], F16, tag="bbb")
            xq_c = xpool.tile([P, DT, TC], F16, tag="xq")
            if c == 0:
                # DMA order tracks chunk-0 consumption: k-phase inputs, then
                # v-phase, then q-phase, then wo (not needed until chunk 1).
                nc.sync.dma_start(out=xk_c[:, 0:4, :], in_=xk[:, 0, 0:4, :])
                nc.sync.dma_start(out=bk_sb[:], in_=bk)
                nc.sync.dma_start(out=xk_c[:, 4:8, :], in_=xk[:, 0, 4:8, :])
                nc.sync.dma_start(out=wv_sb[:],
                                  in_=wv.rearrange("(dt p) j -> p dt j", p=P))
                nc.sync.dma_start(out=xv_c[:, 0:4, :], in_=xv[:, 0, 0:4, :])
                nc.sync.dma_start(out=bv_sb[:], in_=bv)
                nc.sync.dma_start(out=xv_c[:, 4:8, :], in_=xv[:, 0, 4:8, :])
                nc.sync.dma_start(out=bb_c[:], in_=bbb[:, 0, :, :])
                nc.sync.dma_start(out=wq_sb[:],
                                  in_=wq.rearrange("(dt p) j -> p dt j", p=P))
                nc.sync.dma_start(out=bq_sb[:], in_=bq)
                nc.sync.dma_start(out=xq_c[:, 0:4, :], in_=xq[:, 0, 0:4, :])
                nc.sync.dma_start(out=xq_c[:, 4:8, :], in_=xq[:, 0, 4:8, :])
                nc.sync.dma_start(out=wo_sb[:],
                                  in_=wo.rearrange("(jt p) o -> p jt o", p=P))
                # phase-split chunk 0: all k, then all v+scan, then all q.
                # k and v run d-MAJOR (4 lane-tile matmuls per arriving
                # d-slice of x) so PE consumption tracks DMA arrival instead
                # of bursting a full lane-tile ahead of the stream.
                psks = []
                for _lt in range(JT):
                    psk0 = pproj.tile([P, TC], F32, tag="proj")
                    psks.append(psk0)
                for d in range(DT):
                    for lt in range(JT):
                        nc.tensor.matmul(
                            out=psks[lt][:], lhsT=wk_sb[:, d, ts(lt, P)],
                            rhs=xk_c[:, d, :],
                            start=(d == 0), stop=(d == DT - 1),
                        )
                pks = []
                for lt in range(JT):
                    rk = ipool.tile([P, TC], F16, tag="relu")
                    nc.scalar.activation(out=rk[:], in_=psks[lt][:],
                                         func=AF.Relu, bias=bk_sb[:, lt:lt + 1])
                    ek = ipool.tile([P, TC], F16, tag="ex")
                    nc.scalar.activation(out=ek[:], in_=psks[lt][:],
                                         func=AF.Exp, bias=bk_sb[:, lt:lt + 1])
                    pk = ipool.tile([P, TC], F16, tag=f"pk{lt}")
                    nc.vector.scalar_tensor_tensor(
                        out=pk[:], in0=ek[:], scalar=1.0, in1=rk[:],
                        op0=M.min, op1=M.add,
                    )
                    pks.append(pk)
                psvs = []
                for _lt in range(JT):
                    psv0 = pproj.tile([P, TC], F32, tag="proj")
                    psvs.append(psv0)
                for d in range(DT):
                    for lt in range(JT):
                        nc.tensor.matmul(
                            out=psvs[lt][:], lhsT=wv_sb[:, d, ts(lt, P)],
                            rhs=xv_c[:, d, :],
                            start=(d == 0), stop=(d == DT - 1),
                        )
                ss = []
                for lt in range(JT):
                    vsb = ipool.tile([P, TC], F16, tag="vsb")
                    nc.scalar.activation(out=vsb[:], in_=psvs[lt][:],
                                         func=AF.Identity,
                                         bias=bv_sb[:, lt:lt + 1])
                    w = ipool.tile([P, TC], F16, tag="w")
                    nc.vector.tensor_tensor(out=w[:], in0=pks[lt][:],
                                            in1=bb_c[:, lt, :], op=M.mult)
                    g = ipool.tile([P, TC], F16, tag="g")
                    nc.vector.tensor_tensor(out=g[:], in0=pks[lt][:],
                                            in1=w[:], op=M.mult)
                    a = ipool.tile([P, TC], F16, tag="a")
                    nc.vector.tensor_scalar(out=a[:], in0=g[:], scalar1=-1.0,
                                            scalar2=1.0, op0=M.mult, op1=M.add)
                    cc = ipool.tile([P, TC], F16, tag="cc")
                    nc.vector.tensor_tensor(out=cc[:], in0=vsb[:], in1=w[:],
                                            op=M.mult)
                    s_new = spool.tile([P, TC], F16, tag=f"s{lt}")
                    nc.vector.tensor_tensor_scan(
                        out=s_new[:], data0=a[:], data1=cc[:], initial=0.0,
                        op0=M.mult, op1=M.add,
                    )
                    s_prev[lt] = s_new
                    ss.append(s_new)
                y_prev = [emit_q_y(0, lt, xq_c, ss[lt]) for lt in range(JT)]
                continue

            nc.sync.dma_start(out=xk_c[:], in_=xk[:, c, :, :])
            nc.sync.dma_start(out=xv_c[:], in_=xv[:, c, :, :])
            nc.sync.dma_start(out=bb_c[:], in_=bbb[:, c, :, :])
            nc.sync.dma_start(out=xq_c[:], in_=xq[:, c, :, :])

            y_cur = []
            for lt in range(JT):
                pk = emit_k(c, lt, xk_c)
                s_new = emit_v_scan(c, lt, xv_c, bb_c, pk)
                y = emit_q_y(c, lt, xq_c, s_new)
                y_cur.append(y)
                # previous chunk's O-projection, tucked behind lt0's matmuls
                if lt == 0 and y_prev is not None:
                    emit_o_proj(c - 1, y_prev)
                    y_prev = None
            y_prev = y_cur

        emit_o_proj(NCH - 1, y_prev, split_dma=True)

    nc.compile()
    return nc


_NC_CACHE = {}


def _get_nc():
    key = O_COPY_ENG
    if key not in _NC_CACHE:
        _NC_CACHE[key] = build_nc()
    return _NC_CACHE[key]


def make_in_maps(query, key, value, beta, Wq, bq, Wk, bk, Wv, bv, Wb, bb, Wo, bo):
    """Host-side shard prep: core_id = b*2 + hg."""
    ndt = np.float16

    def xpack(x):  # [S, H_DIM] -> [p, chunk, dt, t] in fp16
        a = np.asarray(x, np.float32).T            # [H_DIM, S] = [dt*128+p, c*TC+t]
        a = a.reshape(DT, P, NCH, TC)              # [dt, p, c, t]
        a = a.transpose(1, 2, 0, 3)                # [p, c, dt, t]
        return np.ascontiguousarray(a).astype(ndt)

    def t16(x):
        return np.ascontiguousarray(np.asarray(x, np.float32).T).astype(ndt)

    xqs = [xpack(query[b]) for b in range(B)]
    xks = [xpack(key[b]) for b in range(B)]
    xvs = [xpack(value[b]) for b in range(B)]
    # gate b computed host-side (0.4% of FLOPs), pre-broadcast per lane
    Wbf = np.asarray(Wb, np.float32)
    bbf0 = np.asarray(bb, np.float32)
    z = np.einsum('bsd,hd->bsh', np.asarray(beta, np.float32), Wbf) + bbf0
    bgate = 1.0 / (1.0 + np.exp(-z))                      # [B, S, 16]

    def bpack(bl):  # [S, J] -> [p, chunk, lt, t]
        a = bl.T.reshape(JT, P, NCH, TC)                  # [lt, p, c, t]
        return np.ascontiguousarray(a.transpose(1, 2, 0, 3)).astype(ndt)
    bqf = np.asarray(bq, np.float32)
    bkf = np.asarray(bk, np.float32)
    bvf = np.asarray(bv, np.float32)

    in_maps = []
    for b in range(B):
        for hg in range(HG):
            jsl = slice(hg * J, (hg + 1) * J)
            hsl = slice(hg * HPC, (hg + 1) * HPC)

            def lanes(v):  # [J] -> [128, 4] per lane-tile columns
                return np.ascontiguousarray(v[jsl].reshape(JT, P).T)

            in_maps.append({
                "xq": xqs[b], "xk": xks[b], "xv": xvs[b],
                "bbb": bpack(np.repeat(bgate[b][:, hsl], HEAD_DIM, axis=1)),
                "wq": t16(Wq[jsl]), "wk": t16(Wk[jsl]), "wv": t16(Wv[jsl]),
                "wo": t16(Wo[:, jsl]),
                "bq": lanes(bqf), "bk": lanes(bkf), "bv": lanes(bvf),
            })
    return in_maps


LAST_RESULTS = None


def kernel(**inputs):
    global LAST_RESULTS
    nc = _get_nc()
    in_maps = make_in_maps(**inputs)
    res = run_bass_kernel_spmd(nc, in_maps, core_ids=list(range(NCORES)),
                               trace=bool(os.environ.get("DELTA_TRACE")))
    LAST_RESULTS = res
    bo = np.asarray(inputs["bo"], np.float32)
    out = np.empty((B, S, H_DIM), np.float32)
    for b in range(B):
        m = (res.results[2 * b]["out"].astype(np.float32)
             + res.results[2 * b + 1]["out"].astype(np.float32))
        out[b] = m.T + bo
    return out
